# revision 29
# baseline (speedup 1.0000x reference)
"""Trainium2 Bass kernel for the quirky MultiHeadAttention module.

Reference computation (B=4, S=1024, H=768, NH=12, HS=64):
    Q = (x@Wq+bq)  split into heads     [B,12,S,64]
    K = (x@Wk+bk)  split into heads     [B,12,S,64]
    V = x@Wv+bv    NOT split            [B,S,768]
    A = softmax(QK^T/8 + mask)          [B,12,S,S]
    out = (A @ V) reshaped [B, S*12, H] @ Wo + bo    -> [4, 12288, 768]

Algebraic restructuring used here:
  * (A @ V) @ Wo = A @ (V @ Wo) = A @ (x @ (Wv@Wo) + 1x(bv@Wo)); with bo
    folded in, each output row is A[q,:] @ VW + c, c = bv@Wo + bo, and the
    +c term is realized exactly by adding a constant row to VW (softmax
    rows sum to one in exact correspondence with the sigma column below).
  * Masked keys produce exp(-1e9+s) == 0 in fp32 for every head and every
    query (the mask is [B,1,1,S]), identically in the reference, so masked
    keys are dropped entirely on the host and the key axis is compacted
    (~2x less attention work for a Bernoulli(1/2) mask).
  * The softmax denominator comes from a ones-column appended to VW, and
    exp needs no max-subtraction (scores are O(1) for this problem).

Sharding: 8 cores = 4 batches x 2 head-groups (6 heads each). Pure SPMD,
no collectives. Everything is computed in a transposed layout so no
on-device transposes are needed:
    QT/KT: [384 feat, tok] (head-pairs packed 64+64 in partitions; the
        64-row score matmuls run row-concurrent on the PE)
    S^T = KT_h-slices.T @ QT_h  -> [k, q]  (k on partitions => the mask is
        a per-partition bias folded into the Exp activation for free)
    U = exp(S^T)  [k, q] fp16   -> exactly the layout the PV matmul needs
    O = U.T @ [VW | 1]  -> [q, 769] with col 768 = softmax denominator
Matmul operands are fp16 (full PE rate, ~4x less rounding error than
bf16); all accumulation is fp32 in PSUM.

Schedule: inputs stream over the two HWDGE rings in exactly the order
the tensor engine consumes them (wq + x half-tiles for QT, then wk,
then Wv@Wo), so the PE runs one dense instruction stream: warmup ->
QT (kt-major, paced by arriving x halves) -> KT -> VW with the first
two chunks' score matmuls interleaved from a separate PSUM pool (their
exps drain on the scalar engine meanwhile) -> PV chunks, emitting
chunk c+2's scores after chunk c's PV.  Bias-adds run on the vector
engine and the sigma-half of the output scaling on the scalar engine
so neither blocks PSUM recycling; the HAM clock gate stays open for
the whole kernel (no mid-kernel re-throttle).  The output is written
fp16 (host upcasts to fp32; ~1e-4 extra relative error) to halve the
write traffic and the drain tail.
"""

import math

import numpy as np

B, S, H, NH, HS = 4, 1024, 768, 12, 64
GW = 384          # head-group width = 6 heads * 64
NCORES = 8

_PROGRAM_CACHE = {}


def _pack6(a):
    """[768, N] -> partition-major [128, 6*N] (tile i at cols i*N:(i+1)*N)."""
    n = a.shape[1]
    return np.ascontiguousarray(
        a.reshape(6, 128, n).transpose(1, 0, 2).reshape(128, 6 * n))


def _build_program(kt_tiles, has_cvec):
    """kt_tiles: number of 128-wide compacted-key tiles (1..8).
    has_cvec: include the rank-1 (bv@Wo + bo) constant row in VW."""
    import concourse.mybir as mybir
    import concourse.tile as tile
    from concourse import bacc
    from concourse.bass import ds, ts

    f32 = mybir.dt.float32
    f16 = mybir.dt.float16
    AF = mybir.ActivationFunctionType

    KMAX = 128 * kt_tiles
    # key chunks (<=512 wide, balanced) for the KT projection
    if KMAX <= 512:
        kchunks = [(0, KMAX)]
    else:
        w1 = 128 * ((kt_tiles + 1) // 2)
        kchunks = [(0, w1), (w1, KMAX - w1)]

    nc = bacc.Bacc(None, target_bir_lowering=False, debug=False)

    xp_d = nc.dram_tensor("xp", (128, 6 * 1024), f16, kind="ExternalInput")
    wqp_d = nc.dram_tensor("wqp", (128, 6 * 384), f16, kind="ExternalInput")
    wkp_d = nc.dram_tensor("wkp", (128, 6 * 384), f16, kind="ExternalInput")
    wvp_d = nc.dram_tensor("wvp", (128, 6 * 768), f16, kind="ExternalInput")
    wvo6_d = nc.dram_tensor("wvo6", (1, 768), f16, kind="ExternalInput")
    # small fp32 per-partition vectors: cols = bq(3) bk(3) mk(kt_tiles)
    sv_d = nc.dram_tensor("sv", (128, 6 + kt_tiles), f32, kind="ExternalInput")
    # fp16 output (host upcasts to fp32): halves the write traffic
    out_d = nc.dram_tensor("out", (6, 1024, 768), f16, kind="ExternalOutput")

    with tile.TileContext(nc) as tc:
        with (
            tc.tile_pool(name="persist", bufs=1) as pp,
            tc.tile_pool(name="ut", bufs=6 * kt_tiles) as utp,
            tc.tile_pool(name="eps", bufs=8) as ep,
            tc.tile_pool(name="osb", bufs=4) as op_,
        ):
            # ---- SBUF tiles ----
            sv = pp.tile([128, 6 + kt_tiles], f32, name="sv", tag="sv")
            bq_t = [sv[:, j:j + 1] for j in range(3)]
            bk_t = [sv[:, 3 + j:4 + j] for j in range(3)]
            mk_t = [sv[:, 6 + k:7 + k] for k in range(kt_tiles)]

            xbig = pp.tile([128, 6 * 1024], f16, name="xbig", tag="xbig")
            wqbig = pp.tile([128, 6 * 384], f16, name="wqbig", tag="wqbig")
            wkbig = pp.tile([128, 6 * 384], f16, name="wkbig", tag="wkbig")
            wvbig = pp.tile([128, 6 * 768], f16, name="wvbig", tag="wvbig")
            xkt6 = pp.tile([1, KMAX], f16, name="xkt6", tag="xkt6")
            wvo6 = pp.tile([1, 768], f16, name="wvo6", tag="wvo6")

            xt = [xbig[:, i * 1024:(i + 1) * 1024] for i in range(6)]
            wq_t = [wqbig[:, i * 384:(i + 1) * 384] for i in range(6)]
            # tokens are host-permuted (kept keys first), so the K-side
            # tiles are just the leading columns of the same x buffer
            xkt = [xbig[:, i * 1024:i * 1024 + KMAX] for i in range(6)]
            wk_t = [wkbig[:, i * 384:(i + 1) * 384] for i in range(6)]
            wvo_t = [wvbig[:, i * 768:(i + 1) * 768] for i in range(6)]

            # persistent intermediates
            QT = [pp.tile([128, 1024], f16, name=f"QT{j}", tag=f"QT{j}")
                  for j in range(3)]
            KT = [pp.tile([128, KMAX], f16, name=f"KT{j}", tag=f"KT{j}")
                  for j in range(3)]
            VW = [pp.tile([128, 769], f16, name=f"VW{m}", tag=f"VW{m}")
                  for m in range(kt_tiles)]

            # ---- input DMA, in consumption order ----
            # The tensor engine runs QT -> KT -> scores(0,1) -> VW ->
            # attention chunks, so the stream order is: sv/wq/x tiles
            # (QT), wk (KT), wvp (VW).  Interleaved across the two HWDGE
            # rings; aggregate input is ~4MB (~11us at HBM rate), all of
            # it hidden under phase-A compute.
            # x tiles 0-2 stream as half-tiles (fast QT start); tiles
            # 3-5 as full 262KB pieces - the 2048B per-partition lines
            # double the SDMA descriptor efficiency vs half-tiles, which
            # measured ~45% and starved QT's tail
            wh = 3 * 384
            nc.sync.dma_start(sv[:], sv_d[:])
            nc.scalar.dma_start(wqbig[:, 0:768], wqp_d[:, 0:768])
            for kt in range(3):
                c0 = kt * 1024
                nc.sync.dma_start(xbig[:, c0:c0 + 512], xp_d[:, c0:c0 + 512])
                nc.scalar.dma_start(xbig[:, c0 + 512:c0 + 1024],
                                    xp_d[:, c0 + 512:c0 + 1024])
                if kt == 1:
                    nc.scalar.dma_start(wqbig[:, 768:1536],
                                        wqp_d[:, 768:1536])
            nc.sync.dma_start(xbig[:, 3072:4096], xp_d[:, 3072:4096])
            nc.scalar.dma_start(xbig[:, 4096:5120], xp_d[:, 4096:5120])
            nc.sync.dma_start(xbig[:, 5120:6144], xp_d[:, 5120:6144])
            nc.scalar.dma_start(wqbig[:, 1536:2304], wqp_d[:, 1536:2304])
            nc.sync.dma_start(wkbig[:, 0:wh], wkp_d[:, 0:wh])
            nc.scalar.dma_start(wkbig[:, wh:2 * wh], wkp_d[:, wh:2 * wh])
            for i, ring in enumerate(
                    [nc.sync, nc.scalar, nc.sync, nc.sync, nc.scalar]):
                w0 = [0, 1536, 2304, 3072, 3840][i]
                w1 = [1536, 2304, 3072, 3840, 4608][i]
                ring.dma_start(wvbig[:, w0:w1], wvp_d[:, w0:w1])
            if has_cvec:
                nc.gpsimd.dma_start(wvo6[:], wvo6_d[:])

            nc.vector.memset(xkt6[:], 1.0)

            # ---- phase A ----
            # PE warm-up: dummy matmuls on a tiny memset tile cover the
            # first ~1.5us of input-DMA latency and open the HAM clock
            # gate before the real work arrives.
            # full-width (128-partition) warmup matmuls: HAM's activity
            # monitor ignores near-empty matmuls, so the warmup must load
            # the whole array to open the clock gate (~3.4us of activity)
            wsrc = pp.tile([128, 384], f16, name="wsrc", tag="wsrc")
            nc.vector.memset(wsrc[:], 0.0)
            with tc.tile_pool(name="psW", bufs=1, space="PSUM") as psW:
                psw = psW.tile([128, 384], f32, name="warm", tag="warm")
                for i in range(12):
                    nc.tensor.matmul(psw[:], wsrc[:, 0:128], wsrc[:],
                                     start=(i == 0), stop=(i == 11))

            uts = {}

            def emit_score_pair(ci, kt, pool, tag):
                # both head-halves' matmuls back-to-back so they run
                # row-concurrent on the PE (row groups h0 / h64), then
                # both exps
                j, qc = ci // 2, ci % 2
                pss = []
                for hh in range(2):
                    p0 = hh * 64
                    ps = pool.tile([128, 512], f32, name="psS", tag=tag)
                    nc.tensor.matmul(
                        ps[:],
                        KT[j][p0:p0 + 64, ts(kt, 128)],
                        QT[j][p0:p0 + 64, ds(qc * 512, 512)])
                    pss.append(ps)
                for hh in range(2):
                    u = utp.tile([128, 512], f16, name="ut", tag="ut")
                    nc.scalar.activation(
                        u[:], pss[hh][:], AF.Exp, bias=mk_t[kt])
                    uts[ci][hh][kt] = u

            with tc.tile_pool(name="psA", bufs=6, space="PSUM") as psA:
                # QT projection, kt-major: all six (j,qc) PSUM groups
                # accumulate in parallel so each arriving x tile is
                # consumed immediately (x streams in during QT).
                qgroups = [(j, qc) for qc in range(2) for j in range(3)]
                qps = [psA.tile([128, 512], f32, name=f"qtp{j}{qc}", tag="qk")
                       for j, qc in qgroups]
                for kt in range(6):
                    for gi, (j, qc) in enumerate(qgroups):
                        nc.tensor.matmul(
                            qps[gi][:], wq_t[kt][:, ts(j, 128)],
                            xt[kt][:, ds(qc * 512, 512)],
                            start=(kt == 0), stop=(kt == 5))
                for gi, (j, qc) in enumerate(qgroups):
                    nc.vector.tensor_scalar_add(
                        QT[j][:, ds(qc * 512, 512)], qps[gi][:], bq_t[j])

                # KT projection (wk lands while QT computes)
                for j in range(3):
                    for o, w in kchunks:
                        kch = ds(o, w)
                        ps2 = psA.tile([128, 512], f32, name="ktp", tag="qk")
                        for kt in range(6):
                            nc.tensor.matmul(
                                ps2[:, 0:w], wk_t[kt][:, ts(j, 128)],
                                xkt[kt][:, kch],
                                start=(kt == 0), stop=(kt == 5))
                        nc.vector.tensor_scalar_add(
                            KT[j][:, kch], ps2[:, 0:w], bk_t[j])

            # (psA closes here; VW/scores use their own pools below)
            # VW = x_keys @ (Wv@Wo), m (key-tile) groups 2 at a time
            # (4 PSUM banks), with the first two chunks' score matmuls
            # interleaved from a separate 4-bank pool: their exps drain
            # on the scalar engine in parallel, so the PV chains can
            # start the moment VW completes.
            with (
                tc.tile_pool(name="psV", bufs=6, space="PSUM") as psV,
                tc.tile_pool(name="psSa", bufs=2, space="PSUM") as psSa,
            ):
                squeue = [(ci, kt, hh) for ci in range(2)
                          for kt in range(kt_tiles) for hh in range(2)]
                for ci in range(2):
                    uts[ci] = [[None] * kt_tiles for _ in range(2)]

                def emit_one_score():
                    ci, kt, _ = squeue.pop(0)
                    squeue.pop(0)  # the hh=1 partner of the same (ci, kt)
                    emit_score_pair(ci, kt, psSa, "psS")

                ms = list(range(kt_tiles))
                mchunks = [ms[i:i + 2] for i in range(0, kt_tiles, 2)]
                nrounds = 6 * len(mchunks)
                rd = 0
                for mc in mchunks:
                    vps = {(m, fc): psV.tile([128, 512], f32, name="vw",
                                             tag="vw")
                           for m in mc for fc in range(2)}
                    for kt in range(6):
                        for m in mc:
                            for fc in range(2):
                                nc.tensor.matmul(
                                    vps[(m, fc)][:, 0:384],
                                    xkt[kt][:, ts(m, 128)],
                                    wvo_t[kt][:, ds(fc * 384, 384)],
                                    start=(kt == 0),
                                    stop=(kt == 5 and not has_cvec))
                        rd += 1
                        n0 = 4 * kt_tiles
                        while squeue and n0 - len(squeue) < n0 * rd // nrounds:
                            emit_one_score()
                    for m in mc:
                        for fc in range(2):
                            if has_cvec:
                                nc.tensor.matmul(
                                    vps[(m, fc)][:, 0:384],
                                    xkt6[:, ts(m, 128)],
                                    wvo6[:, ds(fc * 384, 384)],
                                    start=False, stop=True)
                            nc.vector.tensor_copy(
                                VW[m][:, ds(fc * 384, 384)],
                                vps[(m, fc)][:, 0:384])
                    for m in mc:
                        nc.vector.memset(VW[m][:, 768:769], 1.0)
                while squeue:
                    emit_one_score()

            # ---- phase B: attention ----
            with (
                tc.tile_pool(name="psS", bufs=4, space="PSUM") as psSp,
                tc.tile_pool(name="psO", bufs=2, space="PSUM") as psOp,
            ):
                for ci in range(6):
                    j, qc = ci // 2, ci % 2
                    ut = uts.pop(ci)
                    nxt = ci + 2
                    if nxt < 6:
                        uts[nxt] = [[None] * kt_tiles for _ in range(2)]
                    for gi, (hh, mq) in enumerate(
                            (hh, mq) for hh in range(2) for mq in range(4)):
                        head = j * 2 + hh
                        # 384+385 split: both PV chains stream ~160ns/MM so
                        # LDWEIGHTS (~97ns) stays fully hidden; sigma-chain
                        # first so the reciprocal overlaps the other chain.
                        pa = psOp.tile([128, 384], f32, name="psOa",
                                       tag="psOa")
                        pb = psOp.tile([128, 385], f32, name="psOb",
                                       tag="psOb")
                        for kt in range(kt_tiles):
                            nc.tensor.matmul(
                                pb[:], ut[hh][kt][:, ts(mq, 128)],
                                VW[kt][:, 384:769],
                                start=(kt == 0), stop=(kt == kt_tiles - 1))
                        for kt in range(kt_tiles):
                            nc.tensor.matmul(
                                pa[:], ut[hh][kt][:, ts(mq, 128)],
                                VW[kt][:, 0:384],
                                start=(kt == 0), stop=(kt == kt_tiles - 1))
                        rv = ep.tile([128, 1], f32, name="rinv", tag="rinv")
                        nc.vector.reciprocal(rv[:], pb[:, 384:385])
                        ob = op_.tile([128, 768], f16, name="ob", tag="ob")
                        orow = out_d[head, ds(qc * 512 + mq * 128, 128), :]
                        nc.scalar.activation(
                            ob[:, 384:768], pb[:, 0:384], AF.Identity,
                            scale=rv)
                        nc.scalar.dma_start(orow[:, 384:768], ob[:, 384:768])
                        nc.vector.tensor_scalar_mul(
                            ob[:, 0:384], pa[:], rv[:])
                        nc.sync.dma_start(orow[:, 0:384], ob[:, 0:384])
                        # spread chunk (ci+2)'s score pairs between the
                        # PV groups so their exps drain evenly on the
                        # scalar engine instead of piling up at the
                        # chunk boundary
                        if nxt < 6 and 0 <= gi - 2 < min(kt_tiles, 6):
                            emit_score_pair(nxt, gi - 2, psSp, "psS")
                    if nxt < 6:
                        for kt in range(6, kt_tiles):
                            emit_score_pair(nxt, kt, psSp, "psS")
    nc.compile()
    return nc


def get_program(kt_tiles=8, has_cvec=True):
    key = (kt_tiles, has_cvec)
    if key not in _PROGRAM_CACHE:
        _PROGRAM_CACHE[key] = _build_program(*key)
    return _PROGRAM_CACHE[key]


def prep(x, mask, Wq, bq, Wk, bk, Wv, bv, Wo, bo):
    """Host-side sharding/compaction.
    Tokens are permuted per batch so unmasked keys come first; the device
    computes everything in permuted token order and gather_output undoes
    the permutation. Returns (kt_tiles, has_cvec, in_maps, perms)."""
    f16 = np.float16
    x = np.asarray(x, np.float32)
    mask = np.asarray(mask)
    Wq = np.asarray(Wq, np.float32)
    Wk = np.asarray(Wk, np.float32)
    Wv = np.asarray(Wv, np.float32)
    Wo = np.asarray(Wo, np.float32)
    bq = np.asarray(bq, np.float32)
    bk = np.asarray(bk, np.float32)
    bv = np.asarray(bv, np.float32)
    bo = np.asarray(bo, np.float32)

    mrow = [mask[b, 0, 0] != 0 for b in range(B)]
    perms = [np.argsort(~mrow[b], kind="stable") for b in range(B)]
    nkeep = [int(mrow[b].sum()) for b in range(B)]
    kt_tiles = min(8, max(1, math.ceil(max(nkeep) / 128)))
    KMAX = 128 * kt_tiles

    cvec = bv @ Wo + bo
    has_cvec = bool(np.any(cvec))

    # per-head-group packed weights (shared across the 4 batches)
    wq_p, wk_p, bq_p, bk_p = [], [], [], []
    for g in range(2):
        cs = slice(g * GW, (g + 1) * GW)
        wq_p.append(_pack6((Wq[:, cs] * 0.125).astype(f16)))
        wk_p.append(_pack6(Wk[:, cs].astype(f16)))
        bq_p.append((bq[cs] * 0.125).reshape(3, 128).T)   # [128,3]
        bk_p.append(bk[cs].reshape(3, 128).T)
    wvp = _pack6((Wv @ Wo).astype(f16))
    wvo6 = cvec.astype(f16).reshape(1, 768)

    xp_b, sv_b = [], []
    for b in range(B):
        xp_b.append(_pack6(x[b][perms[b]].T.astype(f16)))
        sv = np.empty((128, 6 + kt_tiles), np.float32)
        mk = np.full(KMAX, -1e9, np.float32)
        mk[:nkeep[b]] = 0.0
        sv[:, 6:] = mk.reshape(kt_tiles, 128).T
        sv_b.append(sv)

    in_maps = []
    for c in range(NCORES):
        b, g = c // 2, c % 2
        sv = sv_b[b].copy()
        sv[:, 0:3] = bq_p[g]
        sv[:, 3:6] = bk_p[g]
        in_maps.append({
            "xp": xp_b[b],
            "wqp": wq_p[g],
            "wkp": wk_p[g],
            "wvp": wvp,
            "wvo6": wvo6,
            "sv": sv,
        })
    return kt_tiles, has_cvec, in_maps, perms


def gather_output(results, perms):
    out = np.empty((B, S * NH, H), np.float32)
    ov = out.reshape(B, S, NH, H)
    for c in range(NCORES):
        b, g = c // 2, c % 2
        o = results[c]["out"]  # [6, 1024(permuted q), 768]
        ov[b, perms[b], g * 6:(g + 1) * 6, :] = o.transpose(1, 0, 2)
    return out


def kernel(**inputs):
    from concourse.bass_utils import run_bass_kernel_spmd

    kt_tiles, has_cvec, in_maps, perms = prep(**inputs)
    nc = get_program(kt_tiles, has_cvec)
    res = run_bass_kernel_spmd(nc, in_maps, core_ids=list(range(NCORES)))
    return gather_output(res.results, perms)


if __name__ == "__main__":
    rng = np.random.default_rng(0)
    demo = {
        "x": rng.standard_normal((B, S, H), dtype=np.float32),
        "mask": rng.integers(0, 2, (B, 1, 1, S)).astype(np.int32),
        "Wq": rng.standard_normal((H, H), dtype=np.float32) / np.sqrt(H),
        "bq": np.zeros(H, np.float32),
        "Wk": rng.standard_normal((H, H), dtype=np.float32) / np.sqrt(H),
        "bk": np.zeros(H, np.float32),
        "Wv": rng.standard_normal((H, H), dtype=np.float32) / np.sqrt(H),
        "bv": np.zeros(H, np.float32),
        "Wo": rng.standard_normal((H, H), dtype=np.float32) / np.sqrt(H),
        "bo": np.zeros(H, np.float32),
    }
    out = kernel(**demo)
    print("kernel ran, output shape", out.shape)


# revision 31
# speedup vs baseline: 1.0026x; 1.0026x over previous
"""Trainium2 Bass kernel for the quirky MultiHeadAttention module.

Reference computation (B=4, S=1024, H=768, NH=12, HS=64):
    Q = (x@Wq+bq)  split into heads     [B,12,S,64]
    K = (x@Wk+bk)  split into heads     [B,12,S,64]
    V = x@Wv+bv    NOT split            [B,S,768]
    A = softmax(QK^T/8 + mask)          [B,12,S,S]
    out = (A @ V) reshaped [B, S*12, H] @ Wo + bo    -> [4, 12288, 768]

Algebraic restructuring used here:
  * (A @ V) @ Wo = A @ (V @ Wo) = A @ (x @ (Wv@Wo) + 1x(bv@Wo)); with bo
    folded in, each output row is A[q,:] @ VW + c, c = bv@Wo + bo, and the
    +c term is realized exactly by adding a constant row to VW (softmax
    rows sum to one in exact correspondence with the sigma column below).
  * Masked keys produce exp(-1e9+s) == 0 in fp32 for every head and every
    query (the mask is [B,1,1,S]), identically in the reference, so masked
    keys are dropped entirely on the host and the key axis is compacted
    (~2x less attention work for a Bernoulli(1/2) mask).
  * The softmax denominator comes from a ones-column appended to VW, and
    exp needs no max-subtraction (scores are O(1) for this problem).

Sharding: 8 cores = 4 batches x 2 head-groups (6 heads each). Pure SPMD,
no collectives. Everything is computed in a transposed layout so no
on-device transposes are needed:
    QT/KT: [384 feat, tok] (head-pairs packed 64+64 in partitions; the
        64-row score matmuls run row-concurrent on the PE)
    S^T = KT_h-slices.T @ QT_h  -> [k, q]  (k on partitions => the mask is
        a per-partition bias folded into the Exp activation for free)
    U = exp(S^T)  [k, q] fp16   -> exactly the layout the PV matmul needs
    O = U.T @ [VW | 1]  -> [q, 769] with col 768 = softmax denominator
Matmul operands are fp16 (full PE rate, ~4x less rounding error than
bf16); all accumulation is fp32 in PSUM.

Schedule: inputs stream over the two HWDGE rings in exactly the order
the tensor engine consumes them (wq + x half-tiles for QT, then wk,
then Wv@Wo), so the PE runs one dense instruction stream: warmup ->
QT (kt-major, paced by arriving x halves) -> KT -> VW with the first
two chunks' score matmuls interleaved from a separate PSUM pool (their
exps drain on the scalar engine meanwhile) -> PV chunks, emitting
chunk c+2's scores after chunk c's PV.  Bias-adds run on the vector
engine and the sigma-half of the output scaling on the scalar engine
so neither blocks PSUM recycling; the HAM clock gate stays open for
the whole kernel (no mid-kernel re-throttle).  The output is written
fp16 (host upcasts to fp32; ~1e-4 extra relative error) to halve the
write traffic and the drain tail.
"""

import math

import numpy as np

B, S, H, NH, HS = 4, 1024, 768, 12, 64
GW = 384          # head-group width = 6 heads * 64
NCORES = 8

_PROGRAM_CACHE = {}


def _pack6(a):
    """[768, N] -> partition-major [128, 6*N] (tile i at cols i*N:(i+1)*N)."""
    n = a.shape[1]
    return np.ascontiguousarray(
        a.reshape(6, 128, n).transpose(1, 0, 2).reshape(128, 6 * n))


def _build_program(kt_tiles, has_cvec):
    """kt_tiles: number of 128-wide compacted-key tiles (1..8).
    has_cvec: include the rank-1 (bv@Wo + bo) constant row in VW."""
    import concourse.mybir as mybir
    import concourse.tile as tile
    from concourse import bacc
    from concourse.bass import ds, ts

    f32 = mybir.dt.float32
    f16 = mybir.dt.float16
    AF = mybir.ActivationFunctionType

    KMAX = 128 * kt_tiles
    # key chunks (<=512 wide, balanced) for the KT projection
    if KMAX <= 512:
        kchunks = [(0, KMAX)]
    else:
        w1 = 128 * ((kt_tiles + 1) // 2)
        kchunks = [(0, w1), (w1, KMAX - w1)]

    nc = bacc.Bacc(None, target_bir_lowering=False, debug=False)

    xp_d = nc.dram_tensor("xp", (128, 6 * 1024), f16, kind="ExternalInput")
    wqp_d = nc.dram_tensor("wqp", (128, 6 * 384), f16, kind="ExternalInput")
    wkp_d = nc.dram_tensor("wkp", (128, 6 * 384), f16, kind="ExternalInput")
    wvp_d = nc.dram_tensor("wvp", (128, 6 * 768), f16, kind="ExternalInput")
    wvo6_d = nc.dram_tensor("wvo6", (1, 768), f16, kind="ExternalInput")
    # small fp32 per-partition vectors: cols = bq(3) bk(3) mk(kt_tiles)
    sv_d = nc.dram_tensor("sv", (128, 6 + kt_tiles), f32, kind="ExternalInput")
    # fp16 output (host upcasts to fp32): halves the write traffic
    out_d = nc.dram_tensor("out", (6, 1024, 768), f16, kind="ExternalOutput")

    with tile.TileContext(nc) as tc:
        with (
            tc.tile_pool(name="persist", bufs=1) as pp,
            tc.tile_pool(name="ut", bufs=6 * kt_tiles) as utp,
            tc.tile_pool(name="eps", bufs=8) as ep,
            tc.tile_pool(name="osb", bufs=4) as op_,
        ):
            # ---- SBUF tiles ----
            sv = pp.tile([128, 6 + kt_tiles], f32, name="sv", tag="sv")
            bq_t = [sv[:, j:j + 1] for j in range(3)]
            bk_t = [sv[:, 3 + j:4 + j] for j in range(3)]
            mk_t = [sv[:, 6 + k:7 + k] for k in range(kt_tiles)]

            xbig = pp.tile([128, 6 * 1024], f16, name="xbig", tag="xbig")
            wqbig = pp.tile([128, 6 * 384], f16, name="wqbig", tag="wqbig")
            wkbig = pp.tile([128, 6 * 384], f16, name="wkbig", tag="wkbig")
            wvbig = pp.tile([128, 6 * 768], f16, name="wvbig", tag="wvbig")
            xkt6 = pp.tile([1, KMAX], f16, name="xkt6", tag="xkt6")
            wvo6 = pp.tile([1, 768], f16, name="wvo6", tag="wvo6")

            xt = [xbig[:, i * 1024:(i + 1) * 1024] for i in range(6)]
            wq_t = [wqbig[:, i * 384:(i + 1) * 384] for i in range(6)]
            # tokens are host-permuted (kept keys first), so the K-side
            # tiles are just the leading columns of the same x buffer
            xkt = [xbig[:, i * 1024:i * 1024 + KMAX] for i in range(6)]
            wk_t = [wkbig[:, i * 384:(i + 1) * 384] for i in range(6)]
            wvo_t = [wvbig[:, i * 768:(i + 1) * 768] for i in range(6)]

            # persistent intermediates
            QT = [pp.tile([128, 1024], f16, name=f"QT{j}", tag=f"QT{j}")
                  for j in range(3)]
            KT = [pp.tile([128, KMAX], f16, name=f"KT{j}", tag=f"KT{j}")
                  for j in range(3)]
            VW = [pp.tile([128, 769], f16, name=f"VW{m}", tag=f"VW{m}")
                  for m in range(kt_tiles)]

            # ---- input DMA, in consumption order ----
            # The tensor engine runs QT -> KT -> scores(0,1) -> VW ->
            # attention chunks, so the stream order is: sv/wq/x tiles
            # (QT), wk (KT), wvp (VW).  Interleaved across the two HWDGE
            # rings; aggregate input is ~4MB (~11us at HBM rate), all of
            # it hidden under phase-A compute.
            # x tiles 0-2 stream as half-tiles (fast QT start); tiles
            # 3-5 as full 262KB pieces - the 2048B per-partition lines
            # double the SDMA descriptor efficiency vs half-tiles, which
            # measured ~45% and starved QT's tail
            wh = 3 * 384
            nc.sync.dma_start(sv[:], sv_d[:])
            nc.scalar.dma_start(wqbig[:, 0:768], wqp_d[:, 0:768])
            for kt in range(3):
                c0 = kt * 1024
                nc.sync.dma_start(xbig[:, c0:c0 + 512], xp_d[:, c0:c0 + 512])
                nc.scalar.dma_start(xbig[:, c0 + 512:c0 + 1024],
                                    xp_d[:, c0 + 512:c0 + 1024])
                if kt == 1:
                    nc.scalar.dma_start(wqbig[:, 768:1536],
                                        wqp_d[:, 768:1536])
            nc.sync.dma_start(xbig[:, 3072:4096], xp_d[:, 3072:4096])
            nc.scalar.dma_start(xbig[:, 4096:5120], xp_d[:, 4096:5120])
            nc.sync.dma_start(xbig[:, 5120:6144], xp_d[:, 5120:6144])
            nc.scalar.dma_start(wqbig[:, 1536:2304], wqp_d[:, 1536:2304])
            nc.sync.dma_start(wkbig[:, 0:wh], wkp_d[:, 0:wh])
            nc.scalar.dma_start(wkbig[:, wh:2 * wh], wkp_d[:, wh:2 * wh])
            for i, ring in enumerate(
                    [nc.sync, nc.scalar, nc.sync, nc.sync, nc.scalar]):
                w0 = [0, 1536, 2304, 3072, 3840][i]
                w1 = [1536, 2304, 3072, 3840, 4608][i]
                ring.dma_start(wvbig[:, w0:w1], wvp_d[:, w0:w1])
            if has_cvec:
                nc.gpsimd.dma_start(wvo6[:], wvo6_d[:])

            nc.vector.memset(xkt6[:], 1.0)

            # ---- phase A ----
            # PE warm-up: dummy matmuls on a tiny memset tile cover the
            # first ~1.5us of input-DMA latency and open the HAM clock
            # gate before the real work arrives.
            # full-width (128-partition) warmup matmuls: HAM's activity
            # monitor ignores near-empty matmuls, so the warmup must load
            # the whole array to open the clock gate (~3.4us of activity)
            wsrc = pp.tile([128, 384], f16, name="wsrc", tag="wsrc")
            nc.vector.memset(wsrc[:], 0.0)
            with tc.tile_pool(name="psW", bufs=1, space="PSUM") as psW:
                psw = psW.tile([128, 384], f32, name="warm", tag="warm")
                for i in range(12):
                    nc.tensor.matmul(psw[:], wsrc[:, 0:128], wsrc[:],
                                     start=(i == 0), stop=(i == 11))

            uts = {}

            def emit_score_pair(ci, kt, pool, tag):
                # both head-halves' matmuls back-to-back so they run
                # row-concurrent on the PE (row groups h0 / h64), then
                # both exps
                j, qc = ci // 2, ci % 2
                pss = []
                for hh in range(2):
                    p0 = hh * 64
                    ps = pool.tile([128, 512], f32, name="psS", tag=tag)
                    nc.tensor.matmul(
                        ps[:],
                        KT[j][p0:p0 + 64, ts(kt, 128)],
                        QT[j][p0:p0 + 64, ds(qc * 512, 512)])
                    pss.append(ps)
                for hh in range(2):
                    u = utp.tile([128, 512], f16, name="ut", tag="ut")
                    nc.scalar.activation(
                        u[:], pss[hh][:], AF.Exp, bias=mk_t[kt])
                    uts[ci][hh][kt] = u

            with tc.tile_pool(name="psA", bufs=6, space="PSUM") as psA:
                # QT projection, kt-major: all six (j,qc) PSUM groups
                # accumulate in parallel so each arriving x tile is
                # consumed immediately (x streams in during QT).
                qgroups = [(j, qc) for qc in range(2) for j in range(3)]
                qps = [psA.tile([128, 512], f32, name=f"qtp{j}{qc}", tag="qk")
                       for j, qc in qgroups]
                for kt in range(6):
                    for gi, (j, qc) in enumerate(qgroups):
                        nc.tensor.matmul(
                            qps[gi][:], wq_t[kt][:, ts(j, 128)],
                            xt[kt][:, ds(qc * 512, 512)],
                            start=(kt == 0), stop=(kt == 5))
                for gi, (j, qc) in enumerate(qgroups):
                    nc.vector.tensor_scalar_add(
                        QT[j][:, ds(qc * 512, 512)], qps[gi][:], bq_t[j])

                # KT projection (wk lands while QT computes)
                for j in range(3):
                    for o, w in kchunks:
                        kch = ds(o, w)
                        ps2 = psA.tile([128, 512], f32, name="ktp", tag="qk")
                        for kt in range(6):
                            nc.tensor.matmul(
                                ps2[:, 0:w], wk_t[kt][:, ts(j, 128)],
                                xkt[kt][:, kch],
                                start=(kt == 0), stop=(kt == 5))
                        nc.vector.tensor_scalar_add(
                            KT[j][:, kch], ps2[:, 0:w], bk_t[j])

            # (psA closes here; VW/scores use their own pools below)
            # VW = x_keys @ (Wv@Wo), m (key-tile) groups 2 at a time
            # (4 PSUM banks), with the first two chunks' score matmuls
            # interleaved from a separate 4-bank pool: their exps drain
            # on the scalar engine in parallel, so the PV chains can
            # start the moment VW completes.
            with (
                tc.tile_pool(name="psV", bufs=4, space="PSUM") as psV,
                tc.tile_pool(name="psSa", bufs=4, space="PSUM") as psSa,
            ):
                squeue = [(ci, kt, hh) for ci in range(2)
                          for kt in range(kt_tiles) for hh in range(2)]
                for ci in range(2):
                    uts[ci] = [[None] * kt_tiles for _ in range(2)]

                def emit_one_score():
                    ci, kt, _ = squeue.pop(0)
                    squeue.pop(0)  # the hh=1 partner of the same (ci, kt)
                    emit_score_pair(ci, kt, psSa, "psS")

                ms = list(range(kt_tiles))
                mchunks = [ms[i:i + 2] for i in range(0, kt_tiles, 2)]
                nrounds = 12 * len(mchunks)
                rd = 0
                for mc in mchunks:
                    vps = {(m, fc): psV.tile([128, 512], f32, name="vw",
                                             tag="vw")
                           for m in mc for fc in range(2)}
                    # feature-half passes: fc0's copies are emitted before
                    # fc1's chains, so they drain on the vector engine
                    # while the PE keeps streaming - the next chunk (and
                    # the first PV chain) never waits two copies deep
                    for fc in range(2):
                        for kt in range(6):
                            for m in mc:
                                nc.tensor.matmul(
                                    vps[(m, fc)][:, 0:384],
                                    xkt[kt][:, ts(m, 128)],
                                    wvo_t[kt][:, ds(fc * 384, 384)],
                                    start=(kt == 0),
                                    stop=(kt == 5 and not has_cvec))
                            rd += 1
                            n0 = 4 * kt_tiles
                            while squeue and (
                                    n0 - len(squeue) < n0 * rd // nrounds):
                                emit_one_score()
                        for m in mc:
                            if has_cvec:
                                nc.tensor.matmul(
                                    vps[(m, fc)][:, 0:384],
                                    xkt6[:, ts(m, 128)],
                                    wvo6[:, ds(fc * 384, 384)],
                                    start=False, stop=True)
                            nc.vector.tensor_copy(
                                VW[m][:, ds(fc * 384, 384)],
                                vps[(m, fc)][:, 0:384])
                    for m in mc:
                        nc.vector.memset(VW[m][:, 768:769], 1.0)
                while squeue:
                    emit_one_score()

            # ---- phase B: attention ----
            with (
                tc.tile_pool(name="psS", bufs=4, space="PSUM") as psSp,
                tc.tile_pool(name="psO", bufs=2, space="PSUM") as psOp,
            ):
                for ci in range(6):
                    j, qc = ci // 2, ci % 2
                    ut = uts.pop(ci)
                    nxt = ci + 2
                    if nxt < 6:
                        uts[nxt] = [[None] * kt_tiles for _ in range(2)]
                    for gi, (hh, mq) in enumerate(
                            (hh, mq) for hh in range(2) for mq in range(4)):
                        head = j * 2 + hh
                        # 384+385 split: both PV chains stream ~160ns/MM so
                        # LDWEIGHTS (~97ns) stays fully hidden; sigma-chain
                        # first so the reciprocal overlaps the other chain.
                        pa = psOp.tile([128, 384], f32, name="psOa",
                                       tag="psOa")
                        pb = psOp.tile([128, 385], f32, name="psOb",
                                       tag="psOb")
                        for kt in range(kt_tiles):
                            nc.tensor.matmul(
                                pb[:], ut[hh][kt][:, ts(mq, 128)],
                                VW[kt][:, 384:769],
                                start=(kt == 0), stop=(kt == kt_tiles - 1))
                        for kt in range(kt_tiles):
                            nc.tensor.matmul(
                                pa[:], ut[hh][kt][:, ts(mq, 128)],
                                VW[kt][:, 0:384],
                                start=(kt == 0), stop=(kt == kt_tiles - 1))
                        rv = ep.tile([128, 1], f32, name="rinv", tag="rinv")
                        nc.vector.reciprocal(rv[:], pb[:, 384:385])
                        ob = op_.tile([128, 768], f16, name="ob", tag="ob")
                        orow = out_d[head, ds(qc * 512 + mq * 128, 128), :]
                        nc.scalar.activation(
                            ob[:, 384:768], pb[:, 0:384], AF.Identity,
                            scale=rv)
                        nc.scalar.dma_start(orow[:, 384:768], ob[:, 384:768])
                        nc.vector.tensor_scalar_mul(
                            ob[:, 0:384], pa[:], rv[:])
                        nc.sync.dma_start(orow[:, 0:384], ob[:, 0:384])
                        # spread chunk (ci+2)'s score pairs between the
                        # PV groups so their exps drain evenly on the
                        # scalar engine instead of piling up at the
                        # chunk boundary
                        if nxt < 6 and 0 <= gi - 2 < min(kt_tiles, 6):
                            emit_score_pair(nxt, gi - 2, psSp, "psS")
                    if nxt < 6:
                        for kt in range(6, kt_tiles):
                            emit_score_pair(nxt, kt, psSp, "psS")
    nc.compile()
    return nc


def get_program(kt_tiles=8, has_cvec=True):
    key = (kt_tiles, has_cvec)
    if key not in _PROGRAM_CACHE:
        _PROGRAM_CACHE[key] = _build_program(*key)
    return _PROGRAM_CACHE[key]


def prep(x, mask, Wq, bq, Wk, bk, Wv, bv, Wo, bo):
    """Host-side sharding/compaction.
    Tokens are permuted per batch so unmasked keys come first; the device
    computes everything in permuted token order and gather_output undoes
    the permutation. Returns (kt_tiles, has_cvec, in_maps, perms)."""
    f16 = np.float16
    x = np.asarray(x, np.float32)
    mask = np.asarray(mask)
    Wq = np.asarray(Wq, np.float32)
    Wk = np.asarray(Wk, np.float32)
    Wv = np.asarray(Wv, np.float32)
    Wo = np.asarray(Wo, np.float32)
    bq = np.asarray(bq, np.float32)
    bk = np.asarray(bk, np.float32)
    bv = np.asarray(bv, np.float32)
    bo = np.asarray(bo, np.float32)

    mrow = [mask[b, 0, 0] != 0 for b in range(B)]
    perms = [np.argsort(~mrow[b], kind="stable") for b in range(B)]
    nkeep = [int(mrow[b].sum()) for b in range(B)]
    kt_tiles = min(8, max(1, math.ceil(max(nkeep) / 128)))
    KMAX = 128 * kt_tiles

    cvec = bv @ Wo + bo
    has_cvec = bool(np.any(cvec))

    # per-head-group packed weights (shared across the 4 batches)
    wq_p, wk_p, bq_p, bk_p = [], [], [], []
    for g in range(2):
        cs = slice(g * GW, (g + 1) * GW)
        wq_p.append(_pack6((Wq[:, cs] * 0.125).astype(f16)))
        wk_p.append(_pack6(Wk[:, cs].astype(f16)))
        bq_p.append((bq[cs] * 0.125).reshape(3, 128).T)   # [128,3]
        bk_p.append(bk[cs].reshape(3, 128).T)
    wvp = _pack6((Wv @ Wo).astype(f16))
    wvo6 = cvec.astype(f16).reshape(1, 768)

    xp_b, sv_b = [], []
    for b in range(B):
        xp_b.append(_pack6(x[b][perms[b]].T.astype(f16)))
        sv = np.empty((128, 6 + kt_tiles), np.float32)
        mk = np.full(KMAX, -1e9, np.float32)
        mk[:nkeep[b]] = 0.0
        sv[:, 6:] = mk.reshape(kt_tiles, 128).T
        sv_b.append(sv)

    in_maps = []
    for c in range(NCORES):
        b, g = c // 2, c % 2
        sv = sv_b[b].copy()
        sv[:, 0:3] = bq_p[g]
        sv[:, 3:6] = bk_p[g]
        in_maps.append({
            "xp": xp_b[b],
            "wqp": wq_p[g],
            "wkp": wk_p[g],
            "wvp": wvp,
            "wvo6": wvo6,
            "sv": sv,
        })
    return kt_tiles, has_cvec, in_maps, perms


def gather_output(results, perms):
    out = np.empty((B, S * NH, H), np.float32)
    ov = out.reshape(B, S, NH, H)
    for c in range(NCORES):
        b, g = c // 2, c % 2
        o = results[c]["out"]  # [6, 1024(permuted q), 768]
        ov[b, perms[b], g * 6:(g + 1) * 6, :] = o.transpose(1, 0, 2)
    return out


def kernel(**inputs):
    from concourse.bass_utils import run_bass_kernel_spmd

    kt_tiles, has_cvec, in_maps, perms = prep(**inputs)
    nc = get_program(kt_tiles, has_cvec)
    res = run_bass_kernel_spmd(nc, in_maps, core_ids=list(range(NCORES)))
    return gather_output(res.results, perms)


if __name__ == "__main__":
    rng = np.random.default_rng(0)
    demo = {
        "x": rng.standard_normal((B, S, H), dtype=np.float32),
        "mask": rng.integers(0, 2, (B, 1, 1, S)).astype(np.int32),
        "Wq": rng.standard_normal((H, H), dtype=np.float32) / np.sqrt(H),
        "bq": np.zeros(H, np.float32),
        "Wk": rng.standard_normal((H, H), dtype=np.float32) / np.sqrt(H),
        "bk": np.zeros(H, np.float32),
        "Wv": rng.standard_normal((H, H), dtype=np.float32) / np.sqrt(H),
        "bv": np.zeros(H, np.float32),
        "Wo": rng.standard_normal((H, H), dtype=np.float32) / np.sqrt(H),
        "bo": np.zeros(H, np.float32),
    }
    out = kernel(**demo)
    print("kernel ran, output shape", out.shape)


# revision 32
# speedup vs baseline: 1.0085x; 1.0058x over previous
"""Trainium2 Bass kernel for the quirky MultiHeadAttention module.

Reference computation (B=4, S=1024, H=768, NH=12, HS=64):
    Q = (x@Wq+bq)  split into heads     [B,12,S,64]
    K = (x@Wk+bk)  split into heads     [B,12,S,64]
    V = x@Wv+bv    NOT split            [B,S,768]
    A = softmax(QK^T/8 + mask)          [B,12,S,S]
    out = (A @ V) reshaped [B, S*12, H] @ Wo + bo    -> [4, 12288, 768]

Algebraic restructuring used here:
  * (A @ V) @ Wo = A @ (V @ Wo) = A @ (x @ (Wv@Wo) + 1x(bv@Wo)); with bo
    folded in, each output row is A[q,:] @ VW + c, c = bv@Wo + bo, and the
    +c term is realized exactly by adding a constant row to VW (softmax
    rows sum to one in exact correspondence with the sigma column below).
  * Masked keys produce exp(-1e9+s) == 0 in fp32 for every head and every
    query (the mask is [B,1,1,S]), identically in the reference, so masked
    keys are dropped entirely on the host and the key axis is compacted
    (~2x less attention work for a Bernoulli(1/2) mask).
  * The softmax denominator comes from a ones-column appended to VW, and
    exp needs no max-subtraction (scores are O(1) for this problem).

Sharding: 8 cores = 4 batches x 2 head-groups (6 heads each). Pure SPMD,
no collectives. Everything is computed in a transposed layout so no
on-device transposes are needed:
    QT/KT: [384 feat, tok] (head-pairs packed 64+64 in partitions; the
        64-row score matmuls run row-concurrent on the PE)
    S^T = KT_h-slices.T @ QT_h  -> [k, q]  (k on partitions => the mask is
        a per-partition bias folded into the Exp activation for free)
    U = exp(S^T)  [k, q] fp16   -> exactly the layout the PV matmul needs
    O = U.T @ [VW | 1]  -> [q, 769] with col 768 = softmax denominator
Matmul operands are fp16 (full PE rate, ~4x less rounding error than
bf16); all accumulation is fp32 in PSUM.

Schedule: inputs stream over the two HWDGE rings in exactly the order
the tensor engine consumes them (wq + x half-tiles for QT, then wk,
then Wv@Wo), so the PE runs one dense instruction stream: warmup ->
QT (kt-major, paced by arriving x halves) -> KT -> VW with the first
two chunks' score matmuls interleaved from a separate PSUM pool (their
exps drain on the scalar engine meanwhile) -> PV chunks, emitting
chunk c+2's scores after chunk c's PV.  Bias-adds run on the vector
engine and the sigma-half of the output scaling on the scalar engine
so neither blocks PSUM recycling; the HAM clock gate stays open for
the whole kernel (no mid-kernel re-throttle).  The output is written
fp16 (host upcasts to fp32; ~1e-4 extra relative error) to halve the
write traffic and the drain tail.
"""

import math

import numpy as np

B, S, H, NH, HS = 4, 1024, 768, 12, 64
GW = 384          # head-group width = 6 heads * 64
NCORES = 8

_PROGRAM_CACHE = {}


def _pack6(a):
    """[768, N] -> partition-major [128, 6*N] (tile i at cols i*N:(i+1)*N)."""
    n = a.shape[1]
    return np.ascontiguousarray(
        a.reshape(6, 128, n).transpose(1, 0, 2).reshape(128, 6 * n))


def _build_program(kt_tiles, has_cvec):
    """kt_tiles: number of 128-wide compacted-key tiles (1..8).
    has_cvec: include the rank-1 (bv@Wo + bo) constant row in VW."""
    import concourse.mybir as mybir
    import concourse.tile as tile
    from concourse import bacc
    from concourse.bass import ds, ts

    f32 = mybir.dt.float32
    f16 = mybir.dt.float16
    AF = mybir.ActivationFunctionType

    KMAX = 128 * kt_tiles
    # key chunks (<=512 wide, balanced) for the KT projection
    if KMAX <= 512:
        kchunks = [(0, KMAX)]
    else:
        w1 = 128 * ((kt_tiles + 1) // 2)
        kchunks = [(0, w1), (w1, KMAX - w1)]

    nc = bacc.Bacc(None, target_bir_lowering=False, debug=False)

    xp_d = nc.dram_tensor("xp", (128, 6 * 1024), f16, kind="ExternalInput")
    wqp_d = nc.dram_tensor("wqp", (128, 6 * 384), f16, kind="ExternalInput")
    wkp_d = nc.dram_tensor("wkp", (128, 6 * 384), f16, kind="ExternalInput")
    wvp_d = nc.dram_tensor("wvp", (128, 6 * 768), f16, kind="ExternalInput")
    wvo6_d = nc.dram_tensor("wvo6", (1, 768), f16, kind="ExternalInput")
    # small fp32 per-partition vectors: cols = bq(3) bk(3) mk(kt_tiles)
    sv_d = nc.dram_tensor("sv", (128, 6 + kt_tiles), f32, kind="ExternalInput")
    # fp16 output (host upcasts to fp32): halves the write traffic
    out_d = nc.dram_tensor("out", (6, 1024, 768), f16, kind="ExternalOutput")

    with tile.TileContext(nc) as tc:
        with (
            tc.tile_pool(name="persist", bufs=1) as pp,
            tc.tile_pool(name="ut", bufs=6 * kt_tiles) as utp,
            tc.tile_pool(name="eps", bufs=8) as ep,
            tc.tile_pool(name="osb", bufs=4) as op_,
        ):
            # ---- SBUF tiles ----
            sv = pp.tile([128, 6 + kt_tiles], f32, name="sv", tag="sv")
            bq_t = [sv[:, j:j + 1] for j in range(3)]
            bk_t = [sv[:, 3 + j:4 + j] for j in range(3)]
            mk_t = [sv[:, 6 + k:7 + k] for k in range(kt_tiles)]

            xbig = pp.tile([128, 6 * 1024], f16, name="xbig", tag="xbig")
            wqbig = pp.tile([128, 6 * 384], f16, name="wqbig", tag="wqbig")
            wkbig = pp.tile([128, 6 * 384], f16, name="wkbig", tag="wkbig")
            wvbig = pp.tile([128, 6 * 768], f16, name="wvbig", tag="wvbig")
            xkt6 = pp.tile([1, KMAX], f16, name="xkt6", tag="xkt6")
            wvo6 = pp.tile([1, 768], f16, name="wvo6", tag="wvo6")

            xt = [xbig[:, i * 1024:(i + 1) * 1024] for i in range(6)]
            wq_t = [wqbig[:, i * 384:(i + 1) * 384] for i in range(6)]
            # tokens are host-permuted (kept keys first), so the K-side
            # tiles are just the leading columns of the same x buffer
            xkt = [xbig[:, i * 1024:i * 1024 + KMAX] for i in range(6)]
            wk_t = [wkbig[:, i * 384:(i + 1) * 384] for i in range(6)]
            wvo_t = [wvbig[:, i * 768:(i + 1) * 768] for i in range(6)]

            # persistent intermediates
            QT = [pp.tile([128, 1024], f16, name=f"QT{j}", tag=f"QT{j}")
                  for j in range(3)]
            KT = [pp.tile([128, KMAX], f16, name=f"KT{j}", tag=f"KT{j}")
                  for j in range(3)]
            VW = [pp.tile([128, 769], f16, name=f"VW{m}", tag=f"VW{m}")
                  for m in range(kt_tiles)]

            # ---- input DMA, in consumption order ----
            # The tensor engine runs QT -> KT -> scores(0,1) -> VW ->
            # attention chunks, so the stream order is: sv/wq/x tiles
            # (QT), wk (KT), wvp (VW).  Interleaved across the two HWDGE
            # rings; aggregate input is ~4MB (~11us at HBM rate), all of
            # it hidden under phase-A compute.
            # x tiles 0-2 stream as half-tiles (fast QT start); tiles
            # 3-5 as full 262KB pieces - the 2048B per-partition lines
            # double the SDMA descriptor efficiency vs half-tiles, which
            # measured ~45% and starved QT's tail
            wh = 3 * 384
            nc.sync.dma_start(sv[:], sv_d[:])
            nc.scalar.dma_start(wqbig[:, 0:768], wqp_d[:, 0:768])
            for kt in range(3):
                c0 = kt * 1024
                nc.sync.dma_start(xbig[:, c0:c0 + 512], xp_d[:, c0:c0 + 512])
                nc.scalar.dma_start(xbig[:, c0 + 512:c0 + 1024],
                                    xp_d[:, c0 + 512:c0 + 1024])
                if kt == 1:
                    nc.scalar.dma_start(wqbig[:, 768:1536],
                                        wqp_d[:, 768:1536])
            nc.sync.dma_start(xbig[:, 3072:4096], xp_d[:, 3072:4096])
            nc.scalar.dma_start(xbig[:, 4096:5120], xp_d[:, 4096:5120])
            nc.sync.dma_start(xbig[:, 5120:6144], xp_d[:, 5120:6144])
            nc.scalar.dma_start(wqbig[:, 1536:2304], wqp_d[:, 1536:2304])
            nc.sync.dma_start(wkbig[:, 0:wh], wkp_d[:, 0:wh])
            nc.scalar.dma_start(wkbig[:, wh:2 * wh], wkp_d[:, wh:2 * wh])
            for i, ring in enumerate(
                    [nc.sync, nc.scalar, nc.sync, nc.sync, nc.scalar]):
                w0 = [0, 1536, 2304, 3072, 3840][i]
                w1 = [1536, 2304, 3072, 3840, 4608][i]
                ring.dma_start(wvbig[:, w0:w1], wvp_d[:, w0:w1])
            if has_cvec:
                nc.gpsimd.dma_start(wvo6[:], wvo6_d[:])

            nc.vector.memset(xkt6[:], 1.0)

            # ---- phase A ----
            # PE warm-up: dummy matmuls on a tiny memset tile cover the
            # first ~1.5us of input-DMA latency and open the HAM clock
            # gate before the real work arrives.
            # full-width (128-partition) warmup matmuls: HAM's activity
            # monitor ignores near-empty matmuls, so the warmup must load
            # the whole array to open the clock gate (~3.4us of activity)
            wsrc = pp.tile([128, 384], f16, name="wsrc", tag="wsrc")
            nc.vector.memset(wsrc[:], 0.0)
            with tc.tile_pool(name="psW", bufs=1, space="PSUM") as psW:
                psw = psW.tile([128, 384], f32, name="warm", tag="warm")
                for i in range(12):
                    nc.tensor.matmul(psw[:], wsrc[:, 0:128], wsrc[:],
                                     start=(i == 0), stop=(i == 11))

            uts = {}

            def emit_score_pair(ci, kt, pool, tag):
                # both head-halves' matmuls back-to-back so they run
                # row-concurrent on the PE (row groups h0 / h64), then
                # both exps
                j, qc = ci // 2, ci % 2
                pss = []
                for hh in range(2):
                    p0 = hh * 64
                    ps = pool.tile([128, 512], f32, name="psS", tag=tag)
                    nc.tensor.matmul(
                        ps[:],
                        KT[j][p0:p0 + 64, ts(kt, 128)],
                        QT[j][p0:p0 + 64, ds(qc * 512, 512)])
                    pss.append(ps)
                for hh in range(2):
                    u = utp.tile([128, 512], f16, name="ut", tag="ut")
                    nc.scalar.activation(
                        u[:], pss[hh][:], AF.Exp, bias=mk_t[kt])
                    uts[ci][hh][kt] = u

            with tc.tile_pool(name="psA", bufs=6, space="PSUM") as psA:
                # QT projection, kt-major: all six (j,qc) PSUM groups
                # accumulate in parallel so each arriving x tile is
                # consumed immediately (x streams in during QT).
                qgroups = [(j, qc) for qc in range(2) for j in range(3)]
                qps = [psA.tile([128, 512], f32, name=f"qtp{j}{qc}", tag="qk")
                       for j, qc in qgroups]
                for kt in range(6):
                    for gi, (j, qc) in enumerate(qgroups):
                        nc.tensor.matmul(
                            qps[gi][:], wq_t[kt][:, ts(j, 128)],
                            xt[kt][:, ds(qc * 512, 512)],
                            start=(kt == 0), stop=(kt == 5))
                for gi, (j, qc) in enumerate(qgroups):
                    nc.vector.tensor_scalar_add(
                        QT[j][:, ds(qc * 512, 512)], qps[gi][:], bq_t[j])

                # KT projection (wk lands while QT computes)
                for j in range(3):
                    for o, w in kchunks:
                        kch = ds(o, w)
                        ps2 = psA.tile([128, 512], f32, name="ktp", tag="qk")
                        for kt in range(6):
                            nc.tensor.matmul(
                                ps2[:, 0:w], wk_t[kt][:, ts(j, 128)],
                                xkt[kt][:, kch],
                                start=(kt == 0), stop=(kt == 5))
                        nc.vector.tensor_scalar_add(
                            KT[j][:, kch], ps2[:, 0:w], bk_t[j])

            # (psA closes here; VW/scores use their own pools below)
            # VW = x_keys @ (Wv@Wo), m (key-tile) groups 2 at a time
            # (4 PSUM banks), with the first two chunks' score matmuls
            # interleaved from a separate 4-bank pool: their exps drain
            # on the scalar engine in parallel, so the PV chains can
            # start the moment VW completes.
            with (
                tc.tile_pool(name="psV", bufs=4, space="PSUM") as psV,
                tc.tile_pool(name="psSa", bufs=4, space="PSUM") as psSa,
            ):
                squeue = [(ci, kt, hh) for ci in range(2)
                          for kt in range(kt_tiles) for hh in range(2)]
                for ci in range(2):
                    uts[ci] = [[None] * kt_tiles for _ in range(2)]

                def emit_one_score():
                    ci, kt, _ = squeue.pop(0)
                    squeue.pop(0)  # the hh=1 partner of the same (ci, kt)
                    emit_score_pair(ci, kt, psSa, "psS")

                ms = list(range(kt_tiles))
                mchunks = [ms[i:i + 2] for i in range(0, kt_tiles, 2)]
                nrounds = 6 * len(mchunks)
                rd = 0
                for mc in mchunks:
                    vps = {(m, fc): psV.tile([128, 512], f32, name="vw",
                                             tag="vw")
                           for m in mc for fc in range(2)}
                    for kt in range(6):
                        for m in mc:
                            for fc in range(2):
                                nc.tensor.matmul(
                                    vps[(m, fc)][:, 0:384],
                                    xkt[kt][:, ts(m, 128)],
                                    wvo_t[kt][:, ds(fc * 384, 384)],
                                    start=(kt == 0),
                                    stop=(kt == 5 and not has_cvec))
                        rd += 1
                        n0 = 4 * kt_tiles
                        while squeue and n0 - len(squeue) < n0 * rd // nrounds:
                            emit_one_score()
                    for m in mc:
                        for fc in range(2):
                            if has_cvec:
                                nc.tensor.matmul(
                                    vps[(m, fc)][:, 0:384],
                                    xkt6[:, ts(m, 128)],
                                    wvo6[:, ds(fc * 384, 384)],
                                    start=False, stop=True)
                            nc.vector.tensor_copy(
                                VW[m][:, ds(fc * 384, 384)],
                                vps[(m, fc)][:, 0:384])
                    for m in mc:
                        nc.vector.memset(VW[m][:, 768:769], 1.0)
                while squeue:
                    emit_one_score()

            # ---- phase B: attention ----
            with (
                tc.tile_pool(name="psS", bufs=4, space="PSUM") as psSp,
                tc.tile_pool(name="psO", bufs=2, space="PSUM") as psOp,
            ):
                for ci in range(6):
                    j, qc = ci // 2, ci % 2
                    ut = uts.pop(ci)
                    nxt = ci + 2
                    if nxt < 6:
                        uts[nxt] = [[None] * kt_tiles for _ in range(2)]
                    for gi, (hh, mq) in enumerate(
                            (hh, mq) for hh in range(2) for mq in range(4)):
                        head = j * 2 + hh
                        # 384+385 split: both PV chains stream ~160ns/MM so
                        # LDWEIGHTS (~97ns) stays fully hidden; sigma-chain
                        # first so the reciprocal overlaps the other chain.
                        pa = psOp.tile([128, 384], f32, name="psOa",
                                       tag="psOa")
                        pb = psOp.tile([128, 385], f32, name="psOb",
                                       tag="psOb")
                        for kt in range(kt_tiles):
                            nc.tensor.matmul(
                                pb[:], ut[hh][kt][:, ts(mq, 128)],
                                VW[kt][:, 384:769],
                                start=(kt == 0), stop=(kt == kt_tiles - 1))
                        for kt in range(kt_tiles):
                            nc.tensor.matmul(
                                pa[:], ut[hh][kt][:, ts(mq, 128)],
                                VW[kt][:, 0:384],
                                start=(kt == 0), stop=(kt == kt_tiles - 1))
                        rv = ep.tile([128, 1], f32, name="rinv", tag="rinv")
                        nc.vector.reciprocal(rv[:], pb[:, 384:385])
                        ob = op_.tile([128, 768], f16, name="ob", tag="ob")
                        orow = out_d[head, ds(qc * 512 + mq * 128, 128), :]
                        nc.scalar.activation(
                            ob[:, 384:768], pb[:, 0:384], AF.Identity,
                            scale=rv)
                        nc.scalar.dma_start(orow[:, 384:768], ob[:, 384:768])
                        nc.vector.tensor_scalar_mul(
                            ob[:, 0:384], pa[:], rv[:])
                        nc.sync.dma_start(orow[:, 0:384], ob[:, 0:384])
                        # spread chunk (ci+2)'s score pairs between the
                        # PV groups so their exps drain evenly on the
                        # scalar engine instead of piling up at the
                        # chunk boundary
                        if nxt < 6 and 0 <= gi - 2 < min(kt_tiles, 6):
                            emit_score_pair(nxt, gi - 2, psSp, "psS")
                    if nxt < 6:
                        for kt in range(6, kt_tiles):
                            emit_score_pair(nxt, kt, psSp, "psS")
    nc.compile()
    return nc


def get_program(kt_tiles=8, has_cvec=True):
    key = (kt_tiles, has_cvec)
    if key not in _PROGRAM_CACHE:
        _PROGRAM_CACHE[key] = _build_program(*key)
    return _PROGRAM_CACHE[key]


def prep(x, mask, Wq, bq, Wk, bk, Wv, bv, Wo, bo):
    """Host-side sharding/compaction.
    Tokens are permuted per batch so unmasked keys come first; the device
    computes everything in permuted token order and gather_output undoes
    the permutation. Returns (kt_tiles, has_cvec, in_maps, perms)."""
    f16 = np.float16
    x = np.asarray(x, np.float32)
    mask = np.asarray(mask)
    Wq = np.asarray(Wq, np.float32)
    Wk = np.asarray(Wk, np.float32)
    Wv = np.asarray(Wv, np.float32)
    Wo = np.asarray(Wo, np.float32)
    bq = np.asarray(bq, np.float32)
    bk = np.asarray(bk, np.float32)
    bv = np.asarray(bv, np.float32)
    bo = np.asarray(bo, np.float32)

    mrow = [mask[b, 0, 0] != 0 for b in range(B)]
    perms = [np.argsort(~mrow[b], kind="stable") for b in range(B)]
    nkeep = [int(mrow[b].sum()) for b in range(B)]
    kt_tiles = min(8, max(1, math.ceil(max(nkeep) / 128)))
    KMAX = 128 * kt_tiles

    cvec = bv @ Wo + bo
    has_cvec = bool(np.any(cvec))

    # per-head-group packed weights (shared across the 4 batches)
    wq_p, wk_p, bq_p, bk_p = [], [], [], []
    for g in range(2):
        cs = slice(g * GW, (g + 1) * GW)
        wq_p.append(_pack6((Wq[:, cs] * 0.125).astype(f16)))
        wk_p.append(_pack6(Wk[:, cs].astype(f16)))
        bq_p.append((bq[cs] * 0.125).reshape(3, 128).T)   # [128,3]
        bk_p.append(bk[cs].reshape(3, 128).T)
    wvp = _pack6((Wv @ Wo).astype(f16))
    wvo6 = cvec.astype(f16).reshape(1, 768)

    xp_b, sv_b = [], []
    for b in range(B):
        xp_b.append(_pack6(x[b][perms[b]].T.astype(f16)))
        sv = np.empty((128, 6 + kt_tiles), np.float32)
        mk = np.full(KMAX, -1e9, np.float32)
        mk[:nkeep[b]] = 0.0
        sv[:, 6:] = mk.reshape(kt_tiles, 128).T
        sv_b.append(sv)

    in_maps = []
    for c in range(NCORES):
        b, g = c // 2, c % 2
        sv = sv_b[b].copy()
        sv[:, 0:3] = bq_p[g]
        sv[:, 3:6] = bk_p[g]
        in_maps.append({
            "xp": xp_b[b],
            "wqp": wq_p[g],
            "wkp": wk_p[g],
            "wvp": wvp,
            "wvo6": wvo6,
            "sv": sv,
        })
    return kt_tiles, has_cvec, in_maps, perms


def gather_output(results, perms):
    out = np.empty((B, S * NH, H), np.float32)
    ov = out.reshape(B, S, NH, H)
    for c in range(NCORES):
        b, g = c // 2, c % 2
        o = results[c]["out"]  # [6, 1024(permuted q), 768]
        ov[b, perms[b], g * 6:(g + 1) * 6, :] = o.transpose(1, 0, 2)
    return out


def kernel(**inputs):
    from concourse.bass_utils import run_bass_kernel_spmd

    kt_tiles, has_cvec, in_maps, perms = prep(**inputs)
    nc = get_program(kt_tiles, has_cvec)
    res = run_bass_kernel_spmd(nc, in_maps, core_ids=list(range(NCORES)))
    return gather_output(res.results, perms)


if __name__ == "__main__":
    rng = np.random.default_rng(0)
    demo = {
        "x": rng.standard_normal((B, S, H), dtype=np.float32),
        "mask": rng.integers(0, 2, (B, 1, 1, S)).astype(np.int32),
        "Wq": rng.standard_normal((H, H), dtype=np.float32) / np.sqrt(H),
        "bq": np.zeros(H, np.float32),
        "Wk": rng.standard_normal((H, H), dtype=np.float32) / np.sqrt(H),
        "bk": np.zeros(H, np.float32),
        "Wv": rng.standard_normal((H, H), dtype=np.float32) / np.sqrt(H),
        "bv": np.zeros(H, np.float32),
        "Wo": rng.standard_normal((H, H), dtype=np.float32) / np.sqrt(H),
        "bo": np.zeros(H, np.float32),
    }
    out = kernel(**demo)
    print("kernel ran, output shape", out.shape)
